# revision 15
# baseline (speedup 1.0000x reference)
"""Trainium2 Bass kernel for nn_BaseLoftqLinear (4-bit quantized linear + LoRA + bias).

Computes: out = x @ dequant(W).T + (x @ A.T) @ B.T + bias
  x: [4, 2048, 4096] f32, W: [4096, 4096] 4-bit packed, A: [16, 4096], B: [4096, 16]

Strategy (column-parallel over out_features across 8 cores, transpose-free):
  - each core owns 512 out_features; x replicated
  - host pre-layouts everything so the device does NO PE transposes:
      * x -> bf16, transposed to [IN_F, M] (k-major) so it loads directly as
        matmul lhsT tiles
      * quant nibbles repacked to [IN_F, N/2] so dequant runs in [K, N] layout
      * per-(k,n) scale table expanded on host (c1 folded in)
  - device: dequant W shard + fused B@A into W_eff [K, N] bf16 resident in
    SBUF (DVE/ACT work, overlapped with matmuls), then weight-stationary
    GEMM: po[128m, 512n] += xT[128k,128m].T @ W_eff[128k, 512n] over 32
    k-chunks; bias add on DVE; store f32
  - first 6 m-tiles run k-outer across 6 PSUM banks so PE consumes W_eff
    chunks as they are produced; remaining 58 m-tiles run k-inner
  - host gathers the 8 [8192, 512] outputs -> [4, 2048, 4096]
"""
import os
import sys

for _p in ("/opt/trn_rl_repo", "/root/.axon_site/_ro/trn_rl_repo"):
    if os.path.isdir(_p) and _p not in sys.path:
        sys.path.insert(0, _p)
        break

import numpy as np
import ml_dtypes

import concourse.bass as bass
import concourse.bacc as bacc
import concourse.tile as tile
import concourse.mybir as mybir

dt = mybir.dt

# problem constants (hardcoded per spec)
B_, S_, IN_F, OUT_F, RANK = 4, 2048, 4096, 4096, 16
RANKA = RANK + 64              # lora rank + 64 indicator rows folding delta*scale
M = B_ * S_                    # 8192 tokens
N_CORES = 8
N = OUT_F // N_CORES           # 512 out_features per core
BLOCK = 64                     # quant block size (along in_features)
MT = M // 128                  # 64 m-tiles
KC = IN_F // 128               # 32 k-chunks
NB2 = N // 2                   # 256 packed bytes per k-row (n-pair packing)
GA = 6                         # m-tiles in the k-outer prologue group
SB = 8                         # m-tiles per x superblock
NSB = MT // SB                 # 8 superblocks
XGRP = 4                       # k-chunks per x DMA tile


def build_program(affine: bool, delta: float, lut_vals):
    """Single-core Bass program (SPMD: same program on all 8 cores)."""
    nc = bacc.Bacc("TRN2", target_bir_lowering=False, debug=False,
                   num_devices=N_CORES)

    s_dt = dt.bfloat16 if affine else dt.float32
    xt = nc.dram_tensor("xt", [IN_F, M], dt.bfloat16, kind="ExternalInput")
    pk2 = nc.dram_tensor("pk2", [IN_F, NB2], dt.uint8, kind="ExternalInput")
    sful = nc.dram_tensor("sful", [IN_F, N], s_dt, kind="ExternalInput")
    lora_a = nc.dram_tensor("lora_a", [RANKA, IN_F], dt.bfloat16, kind="ExternalInput")
    lora_bt = nc.dram_tensor("lora_bt", [RANKA, N], dt.bfloat16, kind="ExternalInput")
    bias = nc.dram_tensor("bias", [N], dt.float32, kind="ExternalInput")
    out = nc.dram_tensor("out", [M, N], dt.float32, kind="ExternalOutput")

    with tile.TileContext(nc) as tc:
        with (
            tc.tile_pool(name="const", bufs=1) as constp,
            tc.tile_pool(name="wt", bufs=1) as wtp,
            tc.tile_pool(name="pk", bufs=3) as pkp,
            tc.tile_pool(name="sp", bufs=3) as sp_,
            tc.tile_pool(name="nib", bufs=3) as nibp,
            tc.tile_pool(name="tp", bufs=3) as tp_,
            tc.tile_pool(name="pc", bufs=3) as pcp,
            tc.tile_pool(name="xsb", bufs=16) as xp,
            tc.tile_pool(name="osb", bufs=4) as op_,
            tc.tile_pool(name="ps_ba", bufs=2, space="PSUM") as ps_ba,
            tc.tile_pool(name="ps_out", bufs=GA, space="PSUM") as ps_out,
        ):
            xt_t = xt[:, :].tensor

            def x_dma(g, q):
                """Load x tile (superblock g, chunk-group q): [128k, 4cc x 1024m]."""
                x4 = xp.tile([128, XGRP * SB * 128], dt.bfloat16, tag="x4")
                src = bass.AP(
                    xt_t, (q * XGRP * 128) * M + g * (SB * 128),
                    [[M, 128], [128 * M, XGRP], [1, SB * 128]],
                )
                nc.gpsimd.dma_start(out=x4[:], in_=src)
                return x4

            def x_slice(x4, c, j):
                """lhsT [128k, 128m] for k-chunk c, local m-tile j."""
                ap = x4[:]
                off = (c % XGRP) * (SB * 128) + j * 128
                return bass.AP(ap.tensor, ap.offset + off,
                               [list(ap.ap[0]), [1, 128]])

            # ---- constants first: a/bt gate the very first PE op, so they
            # must not queue behind the bulk x-load burst ----
            a_sb = constp.tile([RANKA, IN_F], dt.bfloat16, name="a_sb")
            nc.sync.dma_start(out=a_sb[:], in_=lora_a[:, :])
            bt_sb = constp.tile([RANKA, N], dt.bfloat16, name="bt_sb")
            nc.sync.dma_start(out=bt_sb[:], in_=lora_bt[:, :])
            bias_sb = constp.tile([128, N], dt.float32, name="bias_sb")
            bsrc = bass.AP(bias[:].tensor, 0, [[0, 128], [1, N]])
            nc.sync.dma_start(out=bias_sb[:], in_=bsrc)

            # ---- x prefetch: only the first 2 chunk-groups up front; the
            # rest stagger through phase 1 so small W-prep DMAs aren't
            # starved by an 8MB burst ----
            xt_tiles = {}
            NQ = KC // XGRP  # 8 chunk-groups per superblock
            for q in range(2):
                xt_tiles[(0, q)] = x_dma(0, q)

            # W_eff resident: wt_sb[:, c*N + nn] = W_eff[c*128 + p, nn]
            wt_sb = wtp.tile([128, KC * N], dt.bfloat16, name="wt_sb")

            po_A = []
            for _j in range(GA):
                poa = ps_out.tile([128, N], dt.float32, tag="po")
                po_A.append(poa)

            # ---- phase 1: W-prep (dequant + B@A) interleaved with group-A ----
            LAG = 2
            prefetched_sb1 = 0

            def emit_groupA(c):
                for j in range(GA):
                    nc.tensor.matmul(
                        po_A[j][:],
                        x_slice(xt_tiles[(0, c // XGRP)], c, j),
                        wt_sb[:, c * N:(c + 1) * N],
                        start=(c == 0), stop=(c == KC - 1),
                    )

            for c in range(KC):
                by = pkp.tile([128, NB2], dt.uint8, tag="by")
                nc.sync.dma_start(out=by[:], in_=pk2[c * 128:(c + 1) * 128, :])
                s_t = sp_.tile([128, N], s_dt, tag="s_t")
                nc.sync.dma_start(out=s_t[:], in_=sful[c * 128:(c + 1) * 128, :])

                # lora + delta fold: pba = (B@A + delta*scale).T chunk
                # [128k, 512n] f32 in PSUM (indicator rows of a_aug select the
                # per-block delta*c1*wmax rows of bt_aug)
                pba = ps_ba.tile([128, N], dt.float32, tag="pba")
                nc.tensor.matmul(
                    pba[:], a_sb[:, c * 128:(c + 1) * 128], bt_sb[:],
                    start=True, stop=True,
                )

                # unpack nibbles: lo -> n<256, hi -> n>=256
                nib = nibp.tile([128, N], dt.uint8, tag="nib")
                nc.vector.tensor_scalar(nib[:, 0:NB2], by[:], 15, None,
                                        mybir.AluOpType.bitwise_and)
                nc.vector.tensor_scalar(nib[:, NB2:N], by[:], 4, None,
                                        mybir.AluOpType.logical_shift_right)

                t = tp_.tile([128, N], dt.bfloat16 if affine else dt.float32,
                             tag="t")
                if affine:
                    # t = idx * scale  (scale = c1 * weight_max from host;
                    # delta*scale arrives via pba)
                    nc.vector.tensor_tensor(t[:], nib[:], s_t[:],
                                            mybir.AluOpType.mult)
                else:
                    # general 16-entry codebook: idx -> sum_k lut[k]*(idx==k)
                    nc.vector.memset(t[:], 0.0)
                    for k in range(16):
                        msk = tp_.tile([128, N], dt.float32, tag="msk")
                        nc.vector.tensor_scalar(msk[:], nib[:], float(k), None,
                                                mybir.AluOpType.is_equal)
                        nc.vector.tensor_scalar_mul(msk[:], msk[:],
                                                    float(lut_vals[k]))
                        nc.vector.tensor_tensor(t[:], t[:], msk[:],
                                                mybir.AluOpType.add)
                    nc.vector.tensor_tensor(t[:], t[:], s_t[:],
                                            mybir.AluOpType.mult)
                # pba PSUM f32 -> SBUF bf16 (on ACT: DVE is the scarce engine)
                pc = pcp.tile([128, N], dt.bfloat16, tag="pc")
                nc.scalar.activation(pc[:], pba[:],
                                     mybir.ActivationFunctionType.Copy,
                                     bias=0.0)
                # W_eff chunk -> resident SBUF (bf16)
                nc.vector.tensor_tensor(wt_sb[:, c * N:(c + 1) * N], t[:],
                                        pc[:], mybir.AluOpType.add)

                # group-A matmuls trail W-prep by LAG chunks
                if c >= LAG:
                    emit_groupA(c - LAG)
                # stagger remaining x loads: sb0 groups 2..7 early, sb1 later
                if c % 2 == 0:
                    if c <= 10:
                        xt_tiles[(0, c // 2 + 2)] = x_dma(0, c // 2 + 2)
                    elif 12 <= c <= 26:
                        xt_tiles[(1, prefetched_sb1)] = x_dma(1, prefetched_sb1)
                        prefetched_sb1 += 1

            for c in range(KC - LAG, KC):
                emit_groupA(c)
            while prefetched_sb1 < NQ:
                xt_tiles[(1, prefetched_sb1)] = x_dma(1, prefetched_sb1)
                prefetched_sb1 += 1

            def store(ms, po):
                o_sb = op_.tile([128, N], dt.float32, tag="o_sb")
                nc.vector.tensor_tensor(o_sb[:], po[:], bias_sb[:],
                                        mybir.AluOpType.add)
                nc.sync.dma_start(out=out[ms * 128:(ms + 1) * 128, :],
                                  in_=o_sb[:])

            for j in range(GA):
                store(j, po_A[j])

            # ---- tail: k-inner m-tiles with resident W_eff ----
            for ms in range(GA, MT):
                g, j = ms // SB, ms % SB
                # prefetch one tile of superblock g+1 per m-tile
                if g + 1 < NSB and (g + 1, j) not in xt_tiles:
                    xt_tiles[(g + 1, j)] = x_dma(g + 1, j)
                po = ps_out.tile([128, N], dt.float32, tag="po")
                for c in range(KC):
                    nc.tensor.matmul(
                        po[:],
                        x_slice(xt_tiles[(g, c // XGRP)], c, j),
                        wt_sb[:, c * N:(c + 1) * N],
                        start=(c == 0), stop=(c == KC - 1),
                    )
                store(ms, po)

    nc.compile()
    return nc


_cache = {}


def _affine_params(lut: np.ndarray):
    lut = np.asarray(lut, dtype=np.float32)
    c1 = float(lut[15] - lut[0]) / 15.0
    idx = np.arange(16, dtype=np.float32)
    affine = bool(
        np.max(np.abs(lut - (lut[0] + c1 * idx))) <= 1e-6 * max(1e-30, np.max(np.abs(lut)))
        and abs(c1) > 1e-20
    )
    delta = float(lut[0]) / c1 if affine else 0.0
    return affine, c1, delta


def _get_program(lut: np.ndarray):
    lut = np.asarray(lut, dtype=np.float32)
    affine, c1, delta = _affine_params(lut)
    key = (affine, round(c1, 12), round(delta, 12), tuple(np.round(lut, 10).tolist()))
    if key not in _cache:
        _cache[key] = build_program(affine, delta, lut.tolist())
    return _cache[key]


def make_in_maps(inputs: dict):
    lut = np.asarray(inputs["lookup_table"], dtype=np.float32)
    affine, c1, delta = _affine_params(lut)
    s_np = ml_dtypes.bfloat16 if affine else np.float32

    x = np.asarray(inputs["x"], dtype=np.float32).reshape(M, IN_F)
    xt = np.ascontiguousarray(x.astype(ml_dtypes.bfloat16).T)  # [IN_F, M]

    pk_full = np.asarray(inputs["packed_qweight"]).astype(np.uint8).reshape(-1)
    idx_full = np.empty(pk_full.size * 2, np.uint8)
    idx_full[0::2] = pk_full & 15
    idx_full[1::2] = pk_full >> 4
    idx_full = idx_full.reshape(OUT_F, IN_F)

    wmax_full = np.asarray(inputs["weight_max"], dtype=np.float32).reshape(OUT_F, IN_F // BLOCK)
    lora_a = np.asarray(inputs["lora_A"], dtype=np.float32)
    lora_b = np.asarray(inputs["lora_B"], dtype=np.float32)
    bias_full = np.asarray(inputs["bias"], dtype=np.float32).reshape(-1)

    # augmented A: lora rows + 64 indicator rows (one per k-block of a chunk
    # pair) so delta*scale folds into the on-device B@A matmul
    NBLK = IN_F // BLOCK  # 64
    a_aug = np.zeros((RANKA, IN_F), np.float32)
    a_aug[:RANK] = lora_a
    for b in range(NBLK):
        a_aug[RANK + b, b * BLOCK:(b + 1) * BLOCK] = 1.0
    a_aug = a_aug.astype(ml_dtypes.bfloat16)

    in_maps = []
    for i in range(N_CORES):
        o0, o1 = i * N, (i + 1) * N
        idxT = idx_full[o0:o1, :].T  # [IN_F, N]
        pk2 = np.ascontiguousarray(idxT[:, :NB2] | (idxT[:, NB2:] << 4))
        wm = wmax_full[o0:o1, :]  # [N, IN_F//BLOCK]
        scale = wm.T * (c1 if affine else 1.0)  # [nblk, N]
        sful = np.ascontiguousarray(
            np.repeat(scale.astype(np.float32), BLOCK, axis=0).astype(s_np))
        bt_aug = np.zeros((RANKA, N), np.float32)
        bt_aug[:RANK] = lora_b[o0:o1].T
        if affine:
            bt_aug[RANK:] = delta * scale  # delta*c1*wmax rows, [nblk, N]
        in_maps.append({
            "xt": xt,
            "pk2": pk2,
            "sful": sful,
            "lora_a": a_aug,
            "lora_bt": np.ascontiguousarray(bt_aug.astype(ml_dtypes.bfloat16)),
            "bias": bias_full[o0:o1],
        })
    return in_maps


def kernel(**inputs) -> np.ndarray:
    from concourse.bass_utils import run_bass_kernel_spmd

    nc = _get_program(inputs["lookup_table"])
    in_maps = make_in_maps(inputs)
    res = run_bass_kernel_spmd(nc, in_maps, core_ids=list(range(N_CORES)))
    outs = [np.asarray(r["out"], dtype=np.float32) for r in res.results]
    full = np.concatenate(outs, axis=1)  # [M, OUT_F]
    return full.reshape(B_, S_, OUT_F)
